# revision 11
# baseline (speedup 1.0000x reference)
"""Causal self-attention (B=2, T=2048, C=1024, H=16) on 8 trn2 NeuronCores.

Sharding: core i handles batch b = i // 4 and head-group hg = i % 4
(4 heads each). Data-parallel over B, tensor-parallel over heads:
each core computes q/k/v for its 4 heads, full causal attention locally,
and a partial projection out = y_heads @ W_proj[rows]; the host sums the
4 partials per batch. No collectives.

Layout trick: everything is computed in "transposed space" so no on-chip
transposes are needed:
  - host passes xT = x[b].T  [C, T]
  - qT/kT [d, T] come straight out of the qkv matmul (W as lhsT, xT as rhs)
  - v [T, d] natural (xT as lhsT, W_v as rhs), augmented with 64 ones
    columns so the y matmul emits the softmax denominator REPLICATED on
    psum partitions 64..127 (no gpsimd partition_broadcast needed)
  - scoresT[k, q] = kT_slice.T @ qT_slice  (contraction over d=64)
  - pT = exp(scale * scoresT)              (ACT, PSUM -> SBUF, bf16)
  - yT[0:64] / denom[64:128] accumulate v_aug.T @ pT over k-tiles
  - y = psy[0:64] * reciprocal(psy[64:128])   (two DVE ops)
  - out[t, c] partial = yT.T @ W_proj_rows  (yT as lhsT, natural W_proj rhs)

All data in bf16 (PSUM accumulation fp32): same PE rate as float32r
(1 col/cycle at >=256 cols) but half the DMA/SBUF traffic. Inputs are
cast host-side; output returns bf16 and is upcast + summed on host.

DMA: weights ride the ACT (scalar) HWDGE queue, x + out ride the SP
(sync) queue, so input streaming and output writeback never serialize
behind each other. wq + x-slice-0 are chunked so the first qkv matmul
starts as soon as chunk 0 lands.
"""

import sys

import numpy as np

sys.path.insert(0, "/opt/trn_rl_repo")

B, T, C = 2, 2048, 1024
N_HEAD = 16
D = C // N_HEAD          # 64
HPC = N_HEAD // 4        # 4 heads per core
CS = HPC * D             # 256 = per-core slice width of q/k/v
NCHUNK = C // 128        # 8 contraction chunks over C
NT = T // 128            # 16 row tiles
NQ = T // 512            # 4 query tiles of 512
SCALE = 1.0 / np.sqrt(D)

_CACHE = {}


def _build():
    import concourse.bacc as bacc
    import concourse.mybir as mybir
    import concourse.tile as tile

    F32 = mybir.dt.float32
    BF16 = mybir.dt.bfloat16

    nc = bacc.Bacc("TRN2", target_bir_lowering=False, debug=False, num_devices=8)

    xT = nc.dram_tensor("xT", [C, T], BF16, kind="ExternalInput").ap()
    wq = nc.dram_tensor("wq", [128, NCHUNK * CS], BF16, kind="ExternalInput").ap()
    wk = nc.dram_tensor("wk", [128, NCHUNK * CS], BF16, kind="ExternalInput").ap()
    wv = nc.dram_tensor("wv", [128, NCHUNK * CS], BF16, kind="ExternalInput").ap()
    wp = nc.dram_tensor("wp", [128, 2 * C], BF16, kind="ExternalInput").ap()
    mask = nc.dram_tensor("mask", [128, 128], BF16, kind="ExternalInput").ap()
    out = nc.dram_tensor("out", [T, C], BF16, kind="ExternalOutput").ap()

    with tile.TileContext(nc) as tc:
        with (
            tc.tile_pool(name="persist", bufs=1) as pp,
            tc.tile_pool(name="consts", bufs=1) as cp,
            tc.tile_pool(name="xw", bufs=1) as xw,
            tc.tile_pool(name="xs", bufs=4) as xsp,
            tc.tile_pool(name="pt", bufs=6) as ptp,
            tc.tile_pool(name="sm", bufs=2) as smp,
            tc.tile_pool(name="po", bufs=4) as pop,
            tc.tile_pool(name="psm", bufs=2, space="PSUM") as psm_p,
            tc.tile_pool(name="psb", bufs=2, space="PSUM") as psb_p,
            tc.tile_pool(name="psy", bufs=2, space="PSUM") as psy_p,
        ):
            # ---------------- persistent SBUF ----------------
            qt = [pp.tile([128, T], BF16, name=f"qt{m}", tag=f"qt{m}")
                  for m in range(2)]
            # Per-head kT, zero-padded to the full 128-partition contraction:
            # head h occupies rows (h%2)*64..+64, the other 64 rows are 0.
            ktp = [pp.tile([128, T], BF16, name=f"ktp{h}", tag=f"ktp{h}")
                   for h in range(HPC)]
            yt = [pp.tile([128, T], BF16, name=f"yt{m}", tag=f"yt{m}")
                  for m in range(2)]
            # v_aug per row-tile: [128, 4 heads, 128] (cols 0..63 = ones, so
            # the y matmul lands the denominator on psum partitions 0..63
            # where the partition-base-0-only custom-DVE reciprocal can
            # read it aligned; cols 64..127 = v data)
            va = [pp.tile([128, HPC, 2 * D], BF16, name=f"va{t}", tag=f"va{t}")
                  for t in range(NT)]
            mk = cp.tile([128, 128], BF16, tag="mask")
            wpt = cp.tile([128, 2, C], BF16, tag="wp")
            zeros_b = cp.tile([64, 512], BF16, tag="zeros")
            warm = cp.tile([128, 1], F32, tag="warm")
            wone = cp.tile([128, 1], F32, tag="wone")

            # ---------------- DMAs, critical-path first ----------------
            wqt = xw.tile([128, NCHUNK, CS], BF16, tag="wq")
            wkt = xw.tile([128, NCHUNK, CS], BF16, tag="wk")
            wvt = xw.tile([128, NCHUNK, CS], BF16, tag="wv")
            # 4 slices, 4 slots (shared tag): every slice stays resident,
            # no WAR coupling between a slice's DMA and an earlier consumer
            xts = [xsp.tile([128, NCHUNK, 512], BF16, name=f"xt{ns}",
                            tag="xt") for ns in range(NQ)]

            def dma_x(ns):
                # x chunks on the SP queue
                for c in range(NCHUNK):
                    nc.sync.dma_start(
                        xts[ns][:, c],
                        xT[c * 128:(c + 1) * 128, ns * 512:(ns + 1) * 512],
                    )

            # weights on the ACT queue; wq chunked for the fast start
            for c in range(NCHUNK):
                nc.scalar.dma_start(
                    wqt[:, c], wq[:, c * CS:(c + 1) * CS])
            dma_x(0)
            nc.scalar.dma_start(wkt[:].rearrange("p c n -> p (c n)"), wk[:])
            nc.scalar.dma_start(wvt[:].rearrange("p c n -> p (c n)"), wv[:])
            dma_x(1)
            nc.scalar.dma_start(wpt[:].rearrange("p c n -> p (c n)"), wp[:])
            nc.scalar.dma_start(mk[:], mask[:])
            dma_x(2)
            dma_x(3)

            nc.gpsimd.memset(zeros_b[:], 0.0)
            nc.gpsimd.memset(wone[:], 1.0)
            # ones columns of v_aug (read by every y_acc; written once)
            for t in range(NT):
                nc.gpsimd.memset(va[t][:, :, 0:D], 1.0)
            # warm the ACT exp table early (off the critical path)
            nc.scalar.activation(warm[:], wone[:],
                                 mybir.ActivationFunctionType.Exp, scale=1.0)
            for h in range(HPC):
                off = 64 - (h % 2) * 64  # the half that stays zero
                for ns in range(NQ):
                    nc.vector.tensor_copy(
                        ktp[h][off:off + 64, ns * 512:(ns + 1) * 512],
                        zeros_b[:],
                    )

            # ---------------- phase B segment: qkv for one 512-col slice ----
            def b_segment(ns):
                xtn = xts[ns]
                sl = slice(ns * 512, (ns + 1) * 512)
                for m in range(2):
                    for w_all, which in ((wqt, "q"), (wkt, "k")):
                        ps = psb_p.tile([128, 512], F32, tag="psb",
                                        name="psqk")
                        for c in range(NCHUNK):
                            nc.tensor.matmul(
                                ps[:],
                                w_all[:, c, m * 128:(m + 1) * 128],
                                xtn[:, c, :],
                                start=(c == 0),
                                stop=(c == NCHUNK - 1),
                            )
                        if which == "q":
                            # ACT is near-idle in qkv-heavy stretches
                            nc.scalar.copy(qt[m][:, sl], ps[:])
                        else:
                            nc.vector.tensor_copy(
                                ktp[2 * m][0:64, sl], ps[0:64, :]
                            )
                            nc.vector.tensor_copy(
                                ktp[2 * m + 1][64:128, sl], ps[64:128, :]
                            )
                for t in range(4 * ns, 4 * ns + 4):
                    ps = psb_p.tile([128, CS], F32, tag="psb", name="psv")
                    for c in range(NCHUNK):
                        nc.tensor.matmul(
                            ps[:],
                            xtn[:, c, (t % 4) * 128:(t % 4 + 1) * 128],
                            wvt[:, c, :],
                            start=(c == 0),
                            stop=(c == NCHUNK - 1),
                        )
                    nc.vector.tensor_copy(
                        va[t][:, :, D:2 * D],
                        ps[:].rearrange("p (h d) -> p h d", h=HPC),
                    )

            # ---------------- attention helpers ----------------
            def scores_exp(h, j, g):
                hq = h // 2
                pss = psm_p.tile([128, 1024], F32, tag="psm", name="pss")
                for i in range(2):
                    kb = g * 2 + i
                    di = kb - 4 * j
                    q0 = 128 * di if di > 0 else 0  # cols < 128*di are fully
                    # masked: never exp-consumed by y (same trim there)
                    nc.tensor.matmul(
                        pss[:, i * 512 + q0:(i + 1) * 512],
                        ktp[h][:, kb * 128:(kb + 1) * 128],
                        qt[hq][:, j * 512 + q0:(j + 1) * 512],
                        start=True,
                        stop=True,
                    )
                pt = ptp.tile([128, 1024], BF16, tag="pt", name="pt")
                if g >= 2 * j:  # diagonal group: exp only the written spans
                    for i in range(2):
                        q0 = 128 * (g * 2 + i - 4 * j)
                        nc.scalar.activation(
                            pt[:, i * 512 + q0:(i + 1) * 512],
                            pss[:, i * 512 + q0:(i + 1) * 512],
                            mybir.ActivationFunctionType.Exp,
                            scale=float(SCALE),
                        )
                else:
                    nc.scalar.activation(
                        pt[:], pss[:],
                        mybir.ActivationFunctionType.Exp,
                        scale=float(SCALE),
                    )
                # zero the upper triangle of the diagonal 128x128 band
                # (columns left of the band are excluded by the y-matmul's
                # trimmed width instead)
                for i in range(2):
                    kb = g * 2 + i
                    di = kb - 4 * j
                    if di >= 0:
                        b0 = i * 512 + 128 * di
                        nc.vector.tensor_mul(
                            pt[:, b0:b0 + 128], pt[:, b0:b0 + 128], mk[:]
                        )
                return pt

            def y_acc(h, j, g, psy, pt):
                nkb = 4 * (j + 1)
                for i in range(2):
                    kb = g * 2 + i
                    di = kb - 4 * j
                    q0 = 128 * di if di > 0 else 0  # cols < 128*di fully masked
                    nc.tensor.matmul(
                        psy[:, q0:512],
                        va[kb][:, h, :],
                        pt[:, i * 512 + q0:(i + 1) * 512],
                        start=(kb == 0),
                        stop=(kb == nkb - 1),
                    )

            def divide(h, j, psy):
                # psy rows 0..63 = denominator replicated, rows 64..127 = y.
                # The custom-DVE reciprocal only works partition-aligned at
                # base 0; tensor_mul tolerates misaligned operands.
                hq, ho = h // 2, (h % 2) * 64
                rec = smp.tile([64, 512], F32, tag="rec", name="rec")
                nc.vector.reciprocal_approx_fast(rec[:], psy[0:D, :])
                nc.vector.tensor_mul(
                    yt[hq][ho:ho + 64, j * 512:(j + 1) * 512],
                    psy[D:2 * D, :],
                    rec[:],
                )

            def attention(j, hp):
                nG = 2 * (j + 1)  # groups of 2 causal k-tiles
                hA, hB = 2 * hp, 2 * hp + 1
                psyA = psy_p.tile([128, 512], F32, tag="psy", name="psyA")
                psyB = psy_p.tile([128, 512], F32, tag="psy", name="psyB")
                ptB_prev = None
                for g in range(nG):
                    ptA = scores_exp(hA, j, g)
                    if ptB_prev is not None:
                        y_acc(hB, j, g - 1, psyB, ptB_prev)
                    yield g
                    ptB = scores_exp(hB, j, g)
                    y_acc(hA, j, g, psyA, ptA)
                    ptB_prev = ptB
                y_acc(hB, j, nG - 1, psyB, ptB_prev)
                divide(hA, j, psyA)
                divide(hB, j, psyB)

            def proj(j, final=False):
                # project rows (t-tiles 4j..4j+3); needs all heads' yt at j
                for t in range(4 * j, 4 * j + 4):
                    for nb in range(2):
                        ps = psb_p.tile([128, 512], F32, tag="psb",
                                        name="pso")
                        for cc in range(2):
                            nc.tensor.matmul(
                                ps[:],
                                yt[cc][:, t * 128:(t + 1) * 128],
                                wpt[:, cc, nb * 512:(nb + 1) * 512],
                                start=(cc == 0),
                                stop=(cc == 1),
                            )
                        ot = pop.tile([128, 512], BF16, tag="po", name="po")
                        # at the tail ACT is idle (exp done) and DVE does the
                        # divides; split the copies so neither engine gates
                        if final and (t + nb) % 2 == 0:
                            nc.scalar.copy(ot[:], ps[:])
                        else:
                            nc.vector.tensor_copy(ot[:], ps[:])
                        # alternate HWDGE queues so the out stream never
                        # serializes on a single ring
                        eng = nc.sync if (t + nb) % 2 == 0 else nc.scalar
                        eng.dma_start(
                            out[t * 128:(t + 1) * 128,
                                nb * 512:(nb + 1) * 512],
                            ot[:],
                        )

            # ---------------- interleaved schedule ----------------
            # B segment ns unlocks attention tasks (j=ns, hp=0/1); proj(j)
            # is emitted two attention-groups into a later task so the
            # divide chain is off the PE's critical path.
            b_segment(0)
            pending_proj = None
            tasks = [(0, 0), (1,), (0, 1), (2,), (1, 0), (3,), (1, 1),
                     (2, 0), (2, 1), (3, 0), (3, 1)]
            for task in tasks:
                if len(task) == 1:
                    b_segment(task[0])
                    continue
                j, hp = task
                for g in attention(j, hp):
                    if g == 1 and pending_proj is not None:
                        proj(pending_proj)
                        pending_proj = None
                if hp == 1:
                    pending_proj = j
            proj(pending_proj, final=True)

    nc.compile()
    return nc


def _causal_mask():
    kk = np.arange(128)[:, None]
    cc = np.arange(128)[None, :]
    return (cc >= kk).astype(np.float32)


def _get_nc():
    if "nc" not in _CACHE:
        _CACHE["nc"] = _build()
    return _CACHE["nc"]


def _run(x, W_qkv, W_proj, trace=False, trace_cores=None):
    import ml_dtypes
    from concourse.bass_utils import run_bass_kernel_spmd

    bf16 = ml_dtypes.bfloat16
    x = np.asarray(x, dtype=np.float32)
    W_qkv = np.asarray(W_qkv, dtype=np.float32)
    W_proj = np.asarray(W_proj, dtype=np.float32)

    nc = _get_nc()
    mask = _causal_mask().astype(bf16)
    in_maps = []
    for core in range(8):
        b, hg = core // 4, core % 4
        sl = slice(hg * CS, (hg + 1) * CS)
        def warr(w):  # [K, N] -> [128, (K//128)*N] chunk-major per partition
            return np.ascontiguousarray(
                w.reshape(w.shape[0] // 128, 128, -1)
                .transpose(1, 0, 2).reshape(128, -1)
            ).astype(bf16)

        in_maps.append({
            "xT": np.ascontiguousarray(x[b].T).astype(bf16),
            "wq": warr(W_qkv[:, sl]),
            "wk": warr(W_qkv[:, C + hg * CS:C + (hg + 1) * CS]),
            "wv": warr(W_qkv[:, 2 * C + hg * CS:2 * C + (hg + 1) * CS]),
            "wp": warr(W_proj[sl, :]),
            "mask": mask,
        })

    res = run_bass_kernel_spmd(
        nc, in_maps, list(range(8)), trace=trace, trace_cores=trace_cores
    )
    outp = np.zeros((B, T, C), dtype=np.float32)
    for core in range(8):
        outp[core // 4] += np.asarray(res.results[core]["out"],
                                      dtype=np.float32)
    return outp, res


def kernel(x, W_qkv, W_proj):
    outp, _ = _run(x, W_qkv, W_proj)
    return outp


# revision 49
# speedup vs baseline: 1.0819x; 1.0819x over previous
"""Causal self-attention (B=2, T=2048, C=1024, H=16) on 8 trn2 NeuronCores.

Sharding: core i handles batch b = i // 4 and head-group hg = i % 4
(4 heads each). Data-parallel over B, tensor-parallel over heads:
each core computes q/k/v for its 4 heads, full causal attention locally,
and a partial projection out = y_heads @ W_proj[rows]; the host sums the
4 partials per batch. No collectives.

Layout trick: everything is computed in "transposed space" so no on-chip
transposes are needed:
  - host passes xT = x[b].T  [C, T]
  - qT/kT [d, T] come straight out of the qkv matmul (W as lhsT, xT as rhs)
  - v [T, d] natural (xT as lhsT, W_v as rhs), augmented with 64 ones
    columns so the y matmul emits the softmax denominator REPLICATED on
    psum partitions 64..127 (no gpsimd partition_broadcast needed)
  - scoresT[k, q] = kT_slice.T @ qT_slice  (contraction over d=64)
  - pT = exp(scale * scoresT)              (ACT, PSUM -> SBUF, bf16)
  - yT[0:64] / denom[64:128] accumulate v_aug.T @ pT over k-tiles
  - y = psy[0:64] * reciprocal(psy[64:128])   (two DVE ops)
  - out[t, c] partial = yT.T @ W_proj_rows  (yT as lhsT, natural W_proj rhs)

All data in bf16 (PSUM accumulation fp32): same PE rate as float32r
(1 col/cycle at >=256 cols) but half the DMA/SBUF traffic. Inputs are
cast host-side; output returns bf16 and is upcast + summed on host.

DMA: weights ride the ACT (scalar) HWDGE queue, x + out ride the SP
(sync) queue, so input streaming and output writeback never serialize
behind each other. wq + x-slice-0 are chunked so the first qkv matmul
starts as soon as chunk 0 lands.
"""

import sys

import numpy as np

sys.path.insert(0, "/opt/trn_rl_repo")

B, T, C = 2, 2048, 1024
N_HEAD = 16
D = C // N_HEAD          # 64
HPC = N_HEAD // 4        # 4 heads per core
CS = HPC * D             # 256 = per-core slice width of q/k/v
NCHUNK = C // 128        # 8 contraction chunks over C
NT = T // 128            # 16 row tiles
NQ = T // 512            # 4 query tiles of 512
SCALE = 1.0 / np.sqrt(D)

_CACHE = {}


def _build():
    import concourse.bacc as bacc
    import concourse.mybir as mybir
    import concourse.tile as tile

    F32 = mybir.dt.float32
    BF16 = mybir.dt.bfloat16

    nc = bacc.Bacc("TRN2", target_bir_lowering=False, debug=False, num_devices=8)

    xT = nc.dram_tensor("xT", [C, T], BF16, kind="ExternalInput").ap()
    wq = nc.dram_tensor("wq", [128, NCHUNK * CS], BF16, kind="ExternalInput").ap()
    wk = nc.dram_tensor("wk", [128, NCHUNK * CS], BF16, kind="ExternalInput").ap()
    wv = nc.dram_tensor("wv", [128, NCHUNK * CS], BF16, kind="ExternalInput").ap()
    wp = nc.dram_tensor("wp", [128, 2 * C], BF16, kind="ExternalInput").ap()
    mask = nc.dram_tensor("mask", [128, 128], BF16, kind="ExternalInput").ap()
    out = nc.dram_tensor("out", [T, C], BF16, kind="ExternalOutput").ap()

    with tile.TileContext(nc) as tc:
        with (
            tc.tile_pool(name="persist", bufs=1) as pp,
            tc.tile_pool(name="consts", bufs=1) as cp,
            tc.tile_pool(name="xw", bufs=1) as xw,
            tc.tile_pool(name="xs", bufs=4) as xsp,
            tc.tile_pool(name="pt", bufs=8) as ptp,
            tc.tile_pool(name="sm", bufs=4) as smp,
            tc.tile_pool(name="po", bufs=6) as pop,
            tc.tile_pool(name="psm", bufs=2, space="PSUM") as psm_p,
            tc.tile_pool(name="psb", bufs=2, space="PSUM") as psb_p,
            tc.tile_pool(name="psy", bufs=2, space="PSUM") as psy_p,
        ):
            # ---------------- persistent SBUF ----------------
            qt = [pp.tile([128, T], BF16, name=f"qt{m}", tag=f"qt{m}")
                  for m in range(2)]
            # Per-head kT, zero-padded to the full 128-partition contraction:
            # head h occupies rows (h%2)*64..+64, the other 64 rows are 0.
            ktp = [pp.tile([128, T], BF16, name=f"ktp{h}", tag=f"ktp{h}")
                   for h in range(HPC)]
            yt = [pp.tile([128, T], BF16, name=f"yt{m}", tag=f"yt{m}")
                  for m in range(2)]
            # v_aug per row-tile: [128, 4 heads, 128] (cols 0..63 = ones, so
            # the y matmul lands the denominator on psum partitions 0..63
            # where the partition-base-0-only custom-DVE reciprocal can
            # read it aligned; cols 64..127 = v data)
            va = [pp.tile([128, HPC, 2 * D], BF16, name=f"va{t}", tag=f"va{t}")
                  for t in range(NT)]
            mk = cp.tile([128, 128], BF16, tag="mask")
            wpt = cp.tile([128, 2, C], BF16, tag="wp")
            zeros_b = cp.tile([64, 512], BF16, tag="zeros")
            warm = cp.tile([128, 1], F32, tag="warm")
            wone = cp.tile([128, 1], F32, tag="wone")

            # ---------------- DMAs, critical-path first ----------------
            wqt = xw.tile([128, NCHUNK, CS], BF16, tag="wq")
            wkt = xw.tile([128, NCHUNK, CS], BF16, tag="wk")
            wvt = xw.tile([128, NCHUNK, CS], BF16, tag="wv")
            # 4 slices, 4 slots (shared tag): every slice stays resident,
            # no WAR coupling between a slice's DMA and an earlier consumer
            xts = [xsp.tile([128, NCHUNK, 512], BF16, name=f"xt{ns}",
                            tag="xt") for ns in range(NQ)]

            def dma_x(ns, eng_for=None, gran=1):
                # x chunk-granular so consumers start as pieces land;
                # eng_for picks the HWDGE queue per granule (default SP).
                # gran>1 groups chunks per dispatch: each dispatch costs
                # ~0.6-1.5us of issuing-engine time, which throttles the
                # early stream more than the transfers themselves.
                for c in range(0, NCHUNK, gran):
                    eng = eng_for(c) if eng_for else nc.sync
                    eng.dma_start(
                        xts[ns][:, c:c + gran],
                        xT[c * 128:(c + gran) * 128,
                           ns * 512:(ns + 1) * 512]
                        .rearrange("(c p) n -> p c n", p=128),
                    )

            # Only the dispatches segment 0's q/k groups depend on go
            # first: each dispatch costs ~600ns of ACT engine time, and a
            # backlog there delays the q copies that gate attention(0).
            # The rest are emitted between segment-0 groups (see below).
            for c in range(0, NCHUNK, 2):
                nc.scalar.dma_start(
                    wqt[:, c:c + 2].rearrange("p c n -> p (c n)"),
                    wq[:, c * CS:(c + 2) * CS])
            dma_x(0, gran=2)
            nc.scalar.dma_start(wkt[:].rearrange("p c n -> p (c n)"), wk[:])

            nc.gpsimd.memset(zeros_b[:], 0.0)
            nc.gpsimd.memset(wone[:], 1.0)
            # ones columns of v_aug (read by every y_acc; written once)
            for t in range(NT):
                nc.gpsimd.memset(va[t][:, :, 0:D], 1.0)
            for h in range(HPC):
                off = 64 - (h % 2) * 64  # the half that stays zero
                for ns in range(NQ):
                    nc.vector.tensor_copy(
                        ktp[h][off:off + 64, ns * 512:(ns + 1) * 512],
                        zeros_b[:],
                    )

            def dma_rest():
                # emitted after segment 0's q groups: these dispatches ride
                # the ACT queue behind the q copies instead of ahead of them
                nc.scalar.dma_start(
                    wvt[:].rearrange("p c n -> p (c n)"), wv[:])
                # x1 feeds the first woven qkv groups (~17us): split it
                # across both HWDGE queues so it lands in time (SWDGE
                # chunk dispatches are far too slow for this)
                dma_x(1, eng_for=lambda c: nc.sync if c % 2 == 0
                      else nc.scalar)
                nc.scalar.dma_start(mk[:], mask[:])
                nc.scalar.dma_start(
                    wpt[:].rearrange("p c n -> p (c n)"), wp[:])
                # x2/x3 arrive far ahead of use: single 3D-AP DMAs via the
                # gpsimd SWDGE — zero HWDGE dispatch slots, third queue
                for ns in (2, 3):
                    nc.gpsimd.dma_start(
                        xts[ns][:],
                        xT[:, ns * 512:(ns + 1) * 512]
                        .rearrange("(c p) n -> p c n", p=128),
                    )
                # warm the ACT exp table (off the critical path)
                nc.scalar.activation(warm[:], wone[:],
                                     mybir.ActivationFunctionType.Exp,
                                     scale=1.0)

            # ---------------- phase B segment: qkv for one 512-col slice ----
            # Generator: yields after each psum group (12 per segment) so
            # the scheduler can weave PE-only qkv work between attention
            # groups, where ACT (exp) is otherwise the pacing engine.
            def b_segment(ns):
                xtn = xts[ns]
                sl = slice(ns * 512, (ns + 1) * 512)
                # q groups first: attention(ns, 0) needs them immediately,
                # while k/v of segment ns are only read from group g=2ns on
                # and can be deferred deep into the attention stream.
                for m in range(2):
                    ps = psb_p.tile([128, 512], F32, tag="psb", name="psq")
                    for c in range(NCHUNK):
                        nc.tensor.matmul(
                            ps[:],
                            wqt[:, c, m * 128:(m + 1) * 128],
                            xtn[:, c, :],
                            start=(c == 0),
                            stop=(c == NCHUNK - 1),
                        )
                    # alternate the copy engine so consecutive psb slots
                    # free in parallel (ACT and DVE each take one)
                    if m == 0:
                        nc.scalar.copy(qt[m][:, sl], ps[:])
                    else:
                        nc.vector.tensor_copy(qt[m][:, sl], ps[:])
                    yield
                for m in range(2):
                    ps = psb_p.tile([128, 512], F32, tag="psb", name="psk")
                    for c in range(NCHUNK):
                        nc.tensor.matmul(
                            ps[:],
                            wkt[:, c, m * 128:(m + 1) * 128],
                            xtn[:, c, :],
                            start=(c == 0),
                            stop=(c == NCHUNK - 1),
                        )
                    nc.vector.tensor_copy(ktp[2 * m][0:64, sl], ps[0:64, :])
                    nc.vector.tensor_copy(
                        ktp[2 * m + 1][64:128, sl], ps[64:128, :]
                    )
                    yield
                for t in range(4 * ns, 4 * ns + 4):
                    ps = psb_p.tile([128, CS], F32, tag="psb", name="psv")
                    for c in range(NCHUNK):
                        nc.tensor.matmul(
                            ps[:],
                            xtn[:, c, (t % 4) * 128:(t % 4 + 1) * 128],
                            wvt[:, c, :],
                            start=(c == 0),
                            stop=(c == NCHUNK - 1),
                        )
                    nc.vector.tensor_copy(
                        va[t][:, :, D:2 * D],
                        ps[:].rearrange("p (h d) -> p h d", h=HPC),
                    )
                    yield

            # ---------------- attention helpers ----------------
            def scores_exp(h, j, g):
                hq = h // 2
                pss = psm_p.tile([128, 1024], F32, tag="psm", name="pss")
                for i in range(2):
                    kb = g * 2 + i
                    di = kb - 4 * j
                    q0 = 128 * di if di > 0 else 0  # cols < 128*di are fully
                    # masked: never exp-consumed by y (same trim there)
                    nc.tensor.matmul(
                        pss[:, i * 512 + q0:(i + 1) * 512],
                        ktp[h][:, kb * 128:(kb + 1) * 128],
                        qt[hq][:, j * 512 + q0:(j + 1) * 512],
                        start=True,
                        stop=True,
                    )
                pt = ptp.tile([128, 1024], BF16, tag="pt", name="pt")
                if g > 2 * j:  # deep diagonal group: exp the written spans
                    for i in range(2):
                        q0 = 128 * (g * 2 + i - 4 * j)
                        nc.scalar.activation(
                            pt[:, i * 512 + q0:(i + 1) * 512],
                            pss[:, i * 512 + q0:(i + 1) * 512],
                            mybir.ActivationFunctionType.Exp,
                            scale=float(SCALE),
                        )
                else:
                    # non-diagonal, and the first diagonal group (g == 2j):
                    # one fused exp; for g == 2j this covers 128 never-read
                    # garbage columns — cheaper than a second ACT dispatch
                    nc.scalar.activation(
                        pt[:], pss[:],
                        mybir.ActivationFunctionType.Exp,
                        scale=float(SCALE),
                    )
                # zero the upper triangle of the diagonal 128x128 band
                # (columns left of the band are excluded by the y-matmul's
                # trimmed width instead)
                for i in range(2):
                    kb = g * 2 + i
                    di = kb - 4 * j
                    if di >= 0:
                        b0 = i * 512 + 128 * di
                        nc.vector.tensor_mul(
                            pt[:, b0:b0 + 128], pt[:, b0:b0 + 128], mk[:]
                        )
                return pt

            def y_acc(h, j, g, psy, pt):
                nkb = 4 * (j + 1)
                for i in range(2):
                    kb = g * 2 + i
                    di = kb - 4 * j
                    q0 = 128 * di if di > 0 else 0  # cols < 128*di fully masked
                    nc.tensor.matmul(
                        psy[:, q0:512],
                        va[kb][:, h, :],
                        pt[:, i * 512 + q0:(i + 1) * 512],
                        start=(kb == 0),
                        stop=(kb == nkb - 1),
                    )

            def divide(h, j, psy, split=1):
                # psy rows 0..63 = denominator replicated, rows 64..127 = y.
                # The custom-DVE reciprocal only works partition-aligned at
                # base 0; tensor_mul tolerates misaligned operands.
                # split>1 chunks the chain so a tail consumer (the final
                # proj) can start on the first 128 columns ~0.8us earlier.
                hq, ho = h // 2, (h % 2) * 64
                w = 512 // split
                for ci in range(split):
                    cs = slice(ci * w, (ci + 1) * w)
                    rec = smp.tile([64, w], F32, tag=f"rec{w}", name="rec")
                    nc.vector.reciprocal_approx_fast(rec[:], psy[0:D, cs])
                    nc.vector.tensor_mul(
                        yt[hq][ho:ho + 64,
                               j * 512 + ci * w:j * 512 + (ci + 1) * w],
                        psy[D:2 * D, cs],
                        rec[:],
                    )

            def attention(j, hp, last=False):
                nG = 2 * (j + 1)  # groups of 2 causal k-tiles
                hA, hB = 2 * hp, 2 * hp + 1
                psyA = psy_p.tile([128, 512], F32, tag="psy", name="psyA")
                psyB = psy_p.tile([128, 512], F32, tag="psy", name="psyB")
                ptB_prev = None
                for g in range(nG):
                    ptA = scores_exp(hA, j, g)
                    if ptB_prev is not None:
                        y_acc(hB, j, g - 1, psyB, ptB_prev)
                    yield g
                    ptB = scores_exp(hB, j, g)
                    y_acc(hA, j, g, psyA, ptA)
                    ptB_prev = ptB
                divide(hA, j, psyA)  # psyA complete; release its bank early
                y_acc(hB, j, nG - 1, psyB, ptB_prev)
                divide(hB, j, psyB, split=4 if last else 1)

            def proj(j, final=False):
                # project rows (t-tiles 4j..4j+3); needs all heads' yt at j
                for t in range(4 * j, 4 * j + 4):
                    ot = pop.tile([128, 2, 512], BF16, tag="po", name="po")
                    for nb in range(2):
                        # final proj: attention is over, the psy slots are
                        # free — alternate psb/psy for 4-deep pipelining
                        pool = (psy_p if final and (t + nb) % 2 else psb_p)
                        ps = pool.tile([128, 512], F32,
                                       tag="psy" if pool is psy_p else "psb",
                                       name="pso")
                        for cc in range(2):
                            nc.tensor.matmul(
                                ps[:],
                                yt[cc][:, t * 128:(t + 1) * 128],
                                wpt[:, cc, nb * 512:(nb + 1) * 512],
                                start=(cc == 0),
                                stop=(cc == 1),
                            )
                        # at the tail ACT is idle (exp done) and DVE does
                        # the divides; split copies so neither engine gates
                        if final and nb == 0:
                            nc.scalar.copy(ot[:, nb], ps[:])
                        else:
                            nc.vector.tensor_copy(ot[:, nb], ps[:])
                    # one [128,1024] DMA per t-tile; alternate HWDGE queues
                    # so the out stream never serializes on a single ring
                    eng = nc.sync if t % 2 == 0 else nc.scalar
                    eng.dma_start(
                        out[t * 128:(t + 1) * 128, :],
                        ot[:].rearrange("p b n -> p (b n)"),
                    )

            # ---------------- interleaved schedule ----------------
            # Segment 0 runs whole (attention(0,·) needs it and it is
            # DMA-paced anyway). Segments 1-3 are fed group-by-group into
            # the attention stream: each qkv group is ~1.7us of PE work
            # with no ACT load, absorbing exp backlog. Feeder ns must
            # drain before attention task (ns, 0) starts. proj(j) is
            # emitted two attention-groups into a later task so the
            # divide chain is off the PE's critical path.
            seg0 = b_segment(0)
            next(seg0)   # q(m0)
            next(seg0)   # q(m1)
            dma_rest()
            for _ in seg0:
                pass
            feeders = {1: b_segment(1), 2: b_segment(2), 3: b_segment(3)}

            def feed(ns, n):
                g = feeders.get(ns)
                if g is None:
                    return
                for _ in range(n):
                    if next(g, "done") == "done":
                        feeders[ns] = None
                        return

            # Feed plan: per task, (segment, groups) consumed at yield g.
            # Deadlines: seg ns q-groups (2) before a(ns,0); k/v groups
            # before g=2*ns of a(ns,·). Spreading qkv into the late,
            # exp-bound tasks keeps the PE fed while ACT drains.
            plan = {
                (0, 1): [(1, 2), (1, 2)],              # f1 q,q,k,k
                (1, 0): [(1, 2), (1, 1), (1, 1)],      # f1 v*4 (by g2)
                (1, 1): [(2, 1), (2, 1), (2, 1), (2, 1)],  # f2 q,q,k,k
                (2, 0): [(2, 2), (2, 1), (2, 1)],      # f2 v*4 (by g4)
                (2, 1): [(3, 1), (3, 1), (3, 1), (3, 1)],  # f3 q,q,k,k
                (3, 0): [(3, 1), (3, 1), (3, 1), (3, 1)],  # f3 v*4 (by g6)
            }
            pending_proj = None
            for task in [(0, 0), (0, 1), (1, 0), (1, 1),
                         (2, 0), (2, 1), (3, 0), (3, 1)]:
                j, hp = task
                steps = plan.get(task, [])
                for g in attention(j, hp, last=(task == (3, 1))):
                    # inject proj into hp==1 tasks: hp==0 tasks already get
                    # the v-group feeds, hp==1 tasks are otherwise exp-bound
                    if g == 1 and hp == 1 and pending_proj is not None:
                        proj(pending_proj)
                        pending_proj = None
                    if g < len(steps):
                        feed(*steps[g])
                if hp == 1:
                    pending_proj = j
            proj(pending_proj, final=True)

    nc.compile()
    return nc


def _causal_mask():
    kk = np.arange(128)[:, None]
    cc = np.arange(128)[None, :]
    return (cc >= kk).astype(np.float32)


def _get_nc():
    if "nc" not in _CACHE:
        _CACHE["nc"] = _build()
    return _CACHE["nc"]


def _run(x, W_qkv, W_proj, trace=False, trace_cores=None):
    import ml_dtypes
    from concourse.bass_utils import run_bass_kernel_spmd

    bf16 = ml_dtypes.bfloat16
    x = np.asarray(x, dtype=np.float32)
    W_qkv = np.asarray(W_qkv, dtype=np.float32)
    W_proj = np.asarray(W_proj, dtype=np.float32)

    nc = _get_nc()
    mask = _causal_mask().astype(bf16)
    in_maps = []
    for core in range(8):
        b, hg = core // 4, core % 4
        sl = slice(hg * CS, (hg + 1) * CS)
        def warr(w):  # [K, N] -> [128, (K//128)*N] chunk-major per partition
            return np.ascontiguousarray(
                w.reshape(w.shape[0] // 128, 128, -1)
                .transpose(1, 0, 2).reshape(128, -1)
            ).astype(bf16)

        in_maps.append({
            "xT": np.ascontiguousarray(x[b].T).astype(bf16),
            "wq": warr(W_qkv[:, sl]),
            "wk": warr(W_qkv[:, C + hg * CS:C + (hg + 1) * CS]),
            "wv": warr(W_qkv[:, 2 * C + hg * CS:2 * C + (hg + 1) * CS]),
            "wp": warr(W_proj[sl, :]),
            "mask": mask,
        })

    res = run_bass_kernel_spmd(
        nc, in_maps, list(range(8)), trace=trace, trace_cores=trace_cores
    )
    outp = np.zeros((B, T, C), dtype=np.float32)
    for core in range(8):
        outp[core // 4] += np.asarray(res.results[core]["out"],
                                      dtype=np.float32)
    return outp, res


def kernel(x, W_qkv, W_proj):
    outp, _ = _run(x, W_qkv, W_proj)
    return outp
